# revision 57
# baseline (speedup 1.0000x reference)
"""Multi-head attention block (QKV proj + softmax attention + out-proj +
residual + LayerNorm) on 8 TRN2 NeuronCores.  ~290us (1.6x over the
460us AllGather/bf16 baseline).

Sharding: core = (batch b, token-half g). Each core computes K/V for ALL 8
heads over the full 2048 tokens of its batch (replicated across the pair —
no collectives; the redundant projections are cheap in fp8 DoubleRow and
the PE slack absorbs them), attention for its 1024 query tokens, then the
output projection, residual and LayerNorm for its token half. Outputs are
disjoint [1024, 1024] shards concatenated on the host.

Inputs are token-rotated per core on the host so rows 0..1023 of the
per-core x are that core's query tokens (softmax over k is permutation-
invariant). The host pre-arranges x^T and all weights in the fp8
DoubleRow pair layout [sc][p, i, n] (contraction index = sc*256+i*128+p)
so every DMA is contiguous 2-4KB rows per partition.

Precision/engine strategy:
- All GEMMs with a >=256-deep contraction (Q/K/V projections, PV, the
  softmax-denominator "ones" matmuls, out-projection) run fp8e4
  DoubleRow (2 contraction rows per PE cell -> 2x MACs/cycle).
  Attention scores stay bf16 (contraction is only 128 = no pairing).
- softmax has no max-subtraction (scores/sqrt(128) ~ N(0, 0.33), exp is
  tiny); weights pt go straight to fp8.  exp is split across TWO
  engines: ScalarE table Exp (fp8 out) for 5/8 of k-chunks, and a DVE
  "Schraudolph" exp for 3/8 — e4m3 bits of exp(y) ~= uint8(y*8*log2e +
  56.5), one tensor_scalar, with PWL error ~ e4m3 quantization and
  under/overflow >9 sigma away.
- K bias dropped exactly (constant over k per query -> softmax
  invariant); V bias and out-proj bias folded on the host into the
  residual via bv @ Wo; the fp32 residual + LayerNorm damp the ~3-5%
  fp8 attention noise to ~7e-4 output error.

Schedule: two query passes of 512 (psum: 3 scores + 2 proj + 1 ctx +
1 sum + 1 y bank). Per (pass, head), 8 super-chunks of 256 k-tokens run
a software pipeline (PV lags scores/exp by one super-chunk); projection
tasks drip-feed into the PE between super-chunks with issue-order
deadlines; pass 1 (exp-bound) absorbs the second-half Q projections and
the out-projections of q-half 0. LayerNorm finishes run in the tail so
the ScalarE Sqrt never thrashes the Exp activation table.
"""

import contextlib
import sys

if '/opt/trn_rl_repo' not in sys.path:
    sys.path.insert(0, '/opt/trn_rl_repo')

import ml_dtypes
import numpy as np

import concourse.bacc as bacc
import concourse.bass as bass
import concourse.bass_utils as bass_utils
import concourse.tile as tile
from concourse import mybir

B, T, D, H = 4, 2048, 1024, 8
DH = 128            # head dim
TQ = T // 2         # query tokens per core
N_CORES = 8
KC = T // 128       # k-token chunks of 128
SC = T // 256       # k-token super-chunks of 256 (DoubleRow pairs) -> 8
NSC = D // 256      # contraction super-chunks for d=1024 -> 4
QC = TQ // 128      # q-token chunks of 128
EPS = 1e-5
ISCALE = 1.0 / float(np.sqrt(DH))
F32 = mybir.dt.float32
BF16 = mybir.dt.bfloat16
FP8 = mybir.dt.float8e4
AF = mybir.ActivationFunctionType
ALU = mybir.AluOpType
DR = mybir.MatmulPerfMode.DoubleRow
BF = ml_dtypes.bfloat16
F8 = ml_dtypes.float8_e4m3fn
# Schraudolph fp8 exp: e4m3 bits of exp(s*ISCALE) ~= s*SCH_A + SCH_B
SCH_A = float(8.0 * np.log2(np.e) * ISCALE)
SCH_B = 56.5


def _body(nc, tc, ap, es, apply_gb):
    xq, xT, Wq, bq, Wk, Wv, Wo, gamma, beta, y = (
        ap['xq'], ap['xT'], ap['Wq'], ap['bq'], ap['Wk'],
        ap['Wv'], ap['Wo'], ap['gamma'], ap['beta'], ap['y'])

    consts = es.enter_context(tc.tile_pool(name="consts", bufs=1))
    xt_pool = es.enter_context(tc.tile_pool(name="xt", bufs=1))
    q_pool = es.enter_context(tc.tile_pool(name="q", bufs=1))
    k_pool = es.enter_context(tc.tile_pool(name="k", bufs=1))
    v_pool = es.enter_context(tc.tile_pool(name="v", bufs=1))
    ctx_pool = es.enter_context(tc.tile_pool(name="ctx", bufs=1))
    wo_pool = es.enter_context(tc.tile_pool(name="wo", bufs=1))
    y1_pool = es.enter_context(tc.tile_pool(name="y1st", bufs=1))
    pt_pool = es.enter_context(tc.tile_pool(name="pt", bufs=2))
    sums_pool = es.enter_context(tc.tile_pool(name="sums", bufs=2))
    xr_pool = es.enter_context(tc.tile_pool(name="xr", bufs=2))
    y3_pool = es.enter_context(tc.tile_pool(name="y3", bufs=2))
    ln_pool = es.enter_context(tc.tile_pool(name="ln", bufs=4))

    # ---- constants -------------------------------------------------------
    # DR ldweights needs the pair-dim stride % 16 == 0, so pad the free dim
    ones2_t = consts.tile([128, 2, 16], FP8, tag="ones2")
    nc.vector.memset(ones2_t, 1.0)
    ones2 = ones2_t[:, :, 0:1]
    eps_t = consts.tile([128, 1], F32, tag="eps")
    nc.vector.memset(eps_t, EPS)

    # partition-broadcast rows (per-feature vectors used on the free dim)
    def bcast128(name, src):
        t = consts.tile([128, D], F32, tag=name, name=name)
        src_b = bass.AP(tensor=src.tensor, offset=src.offset,
                        ap=[[0, 128]] + src.ap)
        nc.sync.dma_start(out=t, in_=src_b)
        return t

    gb = [bcast128("gamma_b", gamma), bcast128("beta_b", beta)] \
        if apply_gb else None

    # per-head bias layout: bias_t[p, h] = b[h*128 + p]
    bq_t = consts.tile([128, H], F32, tag="bq")
    nc.sync.dma_start(out=bq_t, in_=bq.rearrange("(h p) -> p h", p=128))

    # x^T in DoubleRow pair layout: xTp[sc][p, i, t] = xT[sc*256+i*128+p, t]
    # and all weights SBUF-resident in pair layout. DMAs are interleaved
    # sc-wise so the first projection matmuls can start ~3us in instead of
    # waiting for whole tensors.
    xTp = [xt_pool.tile([128, 2, T], FP8, tag=f"xtp{sc}", name=f"xtp{sc}")
           for sc in range(NSC)]
    wq_sb = [wo_pool.tile([128, 2, D], FP8, tag=f"wq{sc}", name=f"wq{sc}")
             for sc in range(NSC)]
    wk_sb = [wo_pool.tile([128, 2, D], FP8, tag=f"wk{sc}", name=f"wk{sc}")
             for sc in range(NSC)]
    wv_sb = [wo_pool.tile([128, 2, D], FP8, tag=f"wv{sc}", name=f"wv{sc}")
             for sc in range(NSC)]

    # K-task inputs first on the sync queue (first PE work), Wq behind
    # them; Wv on the scalar-engine DMA queue in parallel. The host
    # pre-arranges the pair layout, so these are contiguous row DMAs.
    for sc in range(NSC):
        nc.sync.dma_start(out=xTp[sc], in_=xT[sc])
        nc.sync.dma_start(out=wk_sb[sc], in_=Wk[sc])
        nc.scalar.dma_start(out=wv_sb[sc], in_=Wv[sc])
    for sc in range(NSC):
        nc.sync.dma_start(out=wq_sb[sc], in_=Wq[sc])

    # persistent per-head tensors
    KT = [k_pool.tile([128, T], BF16, tag=f"k{h}", name=f"k{h}")
          for h in range(H)]
    QT = [q_pool.tile([128, TQ], BF16, tag=f"q{h}", name=f"q{h}")
          for h in range(H)]
    VT = v_pool.tile([128, H, KC, 128], FP8, tag="vt", name="vt")
    ctx = ctx_pool.tile([128, H, TQ], FP8, tag="ctx")

    # Wo blocks in pair layout (DMA'd mid-attention, used in phase 3)
    wo_sb = [wo_pool.tile([128, 2, D], FP8, tag=f"wo{sc}", name=f"wo{sc}")
             for sc in range(NSC)]

    with contextlib.ExitStack() as es2:
        proj_ps = es2.enter_context(tc.tile_pool(name="proj_ps", bufs=2,
                                                 space="PSUM"))
        scores_ps = es2.enter_context(tc.tile_pool(name="scores_ps", bufs=3,
                                                   space="PSUM"))
        ctx_psum = es2.enter_context(tc.tile_pool(name="ctx_ps", bufs=1,
                                                  space="PSUM"))
        sum_psum = es2.enter_context(tc.tile_pool(name="sum_ps", bufs=1,
                                                  space="PSUM"))
        y_psum = es2.enter_context(tc.tile_pool(name="y_ps", bufs=1,
                                                space="PSUM"))

        # ---- projection tasks (each: 4 DR matmuls + psum-freeing copy) ----
        def q_task(h, nt):
            nsl = slice(nt * 512, (nt + 1) * 512)
            hsl = slice(h * 128, (h + 1) * 128)
            pp = proj_ps.tile([128, 512], F32, tag="pp", name="pp")
            for sc in range(NSC):
                nc.tensor.matmul(pp, wq_sb[sc][:, :, hsl],
                                 xTp[sc][:, :, nsl], perf_mode=DR,
                                 start=(sc == 0), stop=(sc == NSC - 1))
            nc.scalar.add(out=QT[h][:, nsl], in_=pp, add=bq_t[:, h:h + 1])

        def k_task(h, nt):
            nsl = slice(nt * 512, (nt + 1) * 512)
            hsl = slice(h * 128, (h + 1) * 128)
            pp = proj_ps.tile([128, 512], F32, tag="pp", name="pp")
            for sc in range(NSC):
                nc.tensor.matmul(pp, wk_sb[sc][:, :, hsl],
                                 xTp[sc][:, :, nsl], perf_mode=DR,
                                 start=(sc == 0), stop=(sc == NSC - 1))
            nc.scalar.copy(out=KT[h][:, nsl], in_=pp)

        def v_task(blk, fh):
            """V token-major: x-stationary. out psum [128 tok, 512 feats]
            = heads fh*4..fh*4+3; one strided copy into VT."""
            tsl = slice(blk * 128, (blk + 1) * 128)
            fsl = slice(fh * 512, (fh + 1) * 512)
            pp = proj_ps.tile([128, 512], F32, tag="pp", name="pp")
            for sc in range(NSC):
                nc.tensor.matmul(pp, xTp[sc][:, :, tsl],
                                 wv_sb[sc][:, :, fsl], perf_mode=DR,
                                 start=(sc == 0), stop=(sc == NSC - 1))
            nc.vector.tensor_copy(
                out=VT[:, fh * 4:(fh + 1) * 4, blk, :],
                in_=pp.rearrange("p (h m) -> p h m", h=4))

        # ---- out-projection + residual for one 128-q block; the LN finish
        # of the 4 background blocks is deferred to the tail so the ScalarE
        # Sqrt never interleaves with Exp (activation-table thrash)
        y1_st = [y1_pool.tile([128, D], F32, tag=f"y1s{qc}", name=f"y1s{qc}")
                 for qc in range(4)]
        mv_st = ln_pool.tile([128, QC, 2], F32, tag="mv_st")

        def op_mm(qc, y1):
            qs = slice(qc * 128, (qc + 1) * 128)
            xr = xr_pool.tile([128, D], F32, tag="xr")
            nc.sync.dma_start(out=xr, in_=xq[qc * 128:(qc + 1) * 128, :])
            stats = ln_pool.tile([128, 2, 6], F32, tag="stats")
            y1g = y1.rearrange("p (n f) -> p n f", f=512)
            for no in range(D // 512):
                nsl = slice(no * 512, (no + 1) * 512)
                y_ps = y_psum.tile([128, 512], F32, tag="y_ps")
                for sc in range(NSC):
                    nc.tensor.matmul(y_ps,
                                     ctx[:, 2 * sc:2 * sc + 2, qs],
                                     wo_sb[sc][:, :, nsl], perf_mode=DR,
                                     start=(sc == 0), stop=(sc == NSC - 1))
                nc.vector.tensor_add(out=y1[:, nsl], in0=y_ps,
                                     in1=xr[:, nsl])  # resid (+bo)
                nc.vector.bn_stats(out=stats[:, no, :], in_=y1g[:, no, :])
            nc.vector.bn_aggr(out=mv_st[:, qc, :], in_=stats)

        def ln_finish(qc, y1):
            qs = slice(qc * 128, (qc + 1) * 128)
            std = ln_pool.tile([128, 1], F32, tag="std")
            nc.scalar.activation(out=std, in_=mv_st[:, qc, 1:2],
                                 func=AF.Sqrt, bias=eps_t)
            rstd = ln_pool.tile([128, 1], F32, tag="rstd")
            nc.vector.reciprocal(out=rstd, in_=std)
            # y2 = (y1 - mu) * rstd, split across DVE and ScalarE
            # (scalar form: y1*rstd + (-mu*rstd))
            nmr = ln_pool.tile([128, 1], F32, tag="nmr")
            nc.vector.tensor_scalar(out=nmr, in0=rstd,
                                    scalar1=mv_st[:, qc, 0:1], scalar2=-1.0,
                                    op0=ALU.mult, op1=ALU.mult)
            y2 = y3_pool.tile([128, D], F32, tag="y2")
            nc.vector.tensor_scalar(out=y2[:, 0:512], in0=y1[:, 0:512],
                                    scalar1=mv_st[:, qc, 0:1],
                                    scalar2=rstd, op0=ALU.subtract,
                                    op1=ALU.mult)
            nc.scalar.activation(out=y2[:, 512:D], in_=y1[:, 512:D],
                                 func=AF.Identity, scale=rstd, bias=nmr)
            if apply_gb:
                nc.vector.tensor_mul(out=y2, in0=y2, in1=gb[0])
                nc.vector.tensor_add(out=y2, in0=y2, in1=gb[1])
            nc.sync.dma_start(out=y[qs, :], in_=y2)

        # ---- background task schedule (drip-fed into the attention loop).
        # ISSUE-ORDER deadlines (Tile derives deps from program order, so a
        # consumer must be issued after its producer): pass qp / head h
        # scores use K[h] (nt cols) and Q[h][:, qp*512:]; pv(s) (issued in
        # super s) reads VT[.][:, 2s:2s+2, :]; out_proj(qc<4) runs in pass 1
        # and reads every head's pass-0 ctx.
        K = lambda h, nt: (lambda: k_task(h, nt))
        Q = lambda h, nt: (lambda: q_task(h, nt))
        V = lambda b, fh: (lambda: v_task(b, fh))
        OP = lambda qc: (lambda: op_mm(qc, y1_st[qc]))
        bg_step = {
            # pass 0 head 0 pops 3/super; VA pair (2s+2, 2s+3) lands one
            # super ahead of pv(s+1)'s issue; K(0,nt) before super 2nt-1.
            (0, 0): [V(2, 0), V(3, 0), K(0, 1),
                     V(4, 0), V(5, 0), K(0, 2),
                     V(6, 0), V(7, 0), K(0, 3),
                     V(8, 0), V(9, 0), K(1, 0),
                     V(10, 0), V(11, 0), K(1, 1),
                     V(12, 0), V(13, 0), K(1, 2),
                     V(14, 0), V(15, 0), K(1, 3),
                     Q(1, 0), K(2, 0), K(2, 1)],
            (0, 1): [K(2, 2), K(2, 3), Q(2, 0)]
                    + [V(b, 1) for b in range(10)]
                    + [K(3, 0), K(3, 1), K(3, 2)],
            (0, 2): [K(3, 3), Q(3, 0)]
                    + [V(b, 1) for b in range(10, 16)]
                    + [K(4, 0), K(4, 1), K(4, 2), K(4, 3), Q(4, 0),
                       K(5, 0), K(5, 1), K(5, 2)],
            (0, 3): [K(5, 3), Q(5, 0),
                     K(6, 0), K(6, 1), K(6, 2), K(6, 3), Q(6, 0),
                     K(7, 0), K(7, 1), K(7, 2), K(7, 3), Q(7, 0),
                     Q(0, 1)],
            # pass 1 is exp-bound with an idle PE: fill it with the
            # deferred second-half Q projections and the out-projections
            # of the already-complete q-half 0
            (1, 0): [Q(1, 1), OP(0)], (1, 1): [Q(2, 1), OP(1)],
            (1, 2): [Q(3, 1), OP(2)], (1, 3): [Q(4, 1), OP(3)],
            (1, 4): [Q(5, 1)], (1, 5): [Q(6, 1)], (1, 6): [Q(7, 1)],
        }
        bg = []
        bg_i = 0

        def pops(n):
            nonlocal bg_i
            for _ in range(n):
                if bg_i < len(bg):
                    bg[bg_i]()
                    bg_i += 1

        # ---- prologue: just enough for pass 0 head 0 to start ------------
        k_task(0, 0)
        q_task(0, 0)
        v_task(0, 0)
        v_task(1, 0)

        # ---- attention: two query passes of 512 --------------------------
        for qp in range(2):
            qsl = slice(qp * 512, (qp + 1) * 512)
            for h in range(H):
                bg += bg_step.get((qp, h), [])
                pop_rate = 3 if (qp, h) == (0, 0) else (2 if qp == 0 else 1)
                if (qp, h) == (0, 2):
                    for sc in range(NSC):
                        nc.sync.dma_start(out=wo_sb[sc], in_=Wo[sc])

                ctx_ps = ctx_psum.tile([128, 512], F32, tag="ctx_ps")
                sum_ps = sum_psum.tile([1, 512], F32, tag="sum_ps")

                def scores_exp(s):
                    pt = pt_pool.tile([128, 2, 512], FP8, tag="pt", name="pt")
                    for i in range(2):
                        kc = 2 * s + i
                        ks = slice(kc * 128, (kc + 1) * 128)
                        sp = scores_ps.tile([128, 512], F32, tag="sp",
                                            name="sp")
                        nc.tensor.matmul(sp, KT[h][:, ks], QT[h][:, qsl],
                                         start=True, stop=True)
                        if kc % 8 in (1, 3, 5):
                            # DVE "Schraudolph" exp: e4m3 bits of exp(y)
                            # ~= uint8(y*8*log2e + 56.5) (PWL between
                            # powers of 2, error ~ e4m3 quantization)
                            nc.vector.tensor_scalar(
                                out=pt[:, i, :].bitcast(mybir.dt.uint8),
                                in0=sp, scalar1=SCH_A, scalar2=SCH_B,
                                op0=ALU.mult, op1=ALU.add)
                        else:
                            nc.scalar.activation(out=pt[:, i, :], in_=sp,
                                                 func=AF.Exp, scale=ISCALE)
                    return pt

                def pv(s, pt):
                    nc.tensor.matmul(ctx_ps, VT[:, h, 2 * s:2 * s + 2, :],
                                     pt, perf_mode=DR,
                                     start=(s == 0), stop=(s == SC - 1))
                    nc.tensor.matmul(sum_ps, ones2, pt, perf_mode=DR,
                                     start=(s == 0), stop=(s == SC - 1))

                # software pipeline: PV lags scores/exp by one super-chunk
                pt_cur = scores_exp(0)
                for s in range(SC):
                    pt_next = scores_exp(s + 1) if s + 1 < SC else None
                    pv(s, pt_cur)
                    pops(pop_rate)
                    pt_cur = pt_next

                # normalize: ctx[h, qsl] = ctx_ps / sum + bv. The psum
                # tiles are freed by quick copies so the next head's PV /
                # ones chains (same single-buffered banks) don't wait on
                # the full reciprocal+broadcast chain.
                # (bv is folded into xq on the host via bv @ Wo)
                ctxs = sums_pool.tile([128, 512], F32, tag="ctxs")
                nc.vector.tensor_copy(out=ctxs, in_=ctx_ps)
                rsum = sums_pool.tile([1, 512], F32, tag="rsum")
                nc.vector.reciprocal_approx_fast(out=rsum, in_=sum_ps)
                rsum_b = sums_pool.tile([128, 512], F32, tag="rsum_b")
                nc.gpsimd.partition_broadcast(rsum_b, rsum, channels=128)
                nc.vector.tensor_mul(out=ctx[:, h, qsl], in0=ctxs,
                                     in1=rsum_b)

        # ---- tail: remaining out-projection blocks with LN finishes
        # interleaved (all exps are done: one Exp->Sqrt table swap); the
        # deferred finishes of the background blocks fill the gaps
        y1_t = [y3_pool.tile([128, D], F32, tag=f"y1{qc % 2}", name="y1")
                for qc in range(4, QC)]
        for qc in range(4, QC):
            op_mm(qc, y1_t[qc - 4])
            if qc >= 5:
                ln_finish(qc - 5 + 0, y1_st[qc - 5])
        for qc in range(3, 4):
            ln_finish(qc, y1_st[qc])
        for qc in range(4, QC):
            ln_finish(qc, y1_t[qc - 4])


def build(apply_gb=True):
    nc = bacc.Bacc("TRN2", target_bir_lowering=False, debug=False,
                   enable_asserts=False, num_devices=N_CORES)
    ap = {}
    ap['xq'] = nc.dram_tensor("xq", [TQ, D], F32, kind="ExternalInput").ap()
    ap['xT'] = nc.dram_tensor("xT", [NSC, 128, 2, T], FP8,
                              kind="ExternalInput").ap()
    ap['bq'] = nc.dram_tensor("bq", [D], F32, kind="ExternalInput").ap()
    for w in ('Wq', 'Wk', 'Wv', 'Wo'):
        ap[w] = nc.dram_tensor(w, [NSC, 128, 2, D], FP8,
                               kind="ExternalInput").ap()
    ap['gamma'] = nc.dram_tensor("gamma", [D], F32, kind="ExternalInput").ap()
    ap['beta'] = nc.dram_tensor("beta", [D], F32, kind="ExternalInput").ap()
    ap['y'] = nc.dram_tensor("y", [TQ, D], F32, kind="ExternalOutput").ap()

    with tile.TileContext(nc) as tc, contextlib.ExitStack() as es:
        _body(nc, tc, ap, es, apply_gb)
    nc.compile()
    return nc


def make_in_maps(inputs):
    """Per-core input maps; x token-rotated so q tokens come first."""
    f32 = {k: np.ascontiguousarray(np.asarray(v, dtype=np.float32))
           for k, v in inputs.items()}
    shared = {k: f32[k] for k in ('bq', 'gamma', 'beta')}

    def pair4(a):  # [D, N] -> [NSC, 128, 2, N] DoubleRow pair layout
        return np.ascontiguousarray(
            a.astype(F8).reshape(NSC, 2, 128, a.shape[1]).transpose(
                0, 2, 1, 3))

    for w in ('Wq', 'Wk', 'Wv', 'Wo'):
        shared[w] = pair4(f32[w])
    x = f32['x']
    # fold both the out-proj bias and the V bias into the residual:
    # (ctx0 + bv) @ Wo + bo + x = ctx0 @ Wo + (x + bo + bv@Wo)
    resid = f32['bo'] + f32['bv'] @ f32['Wo']
    in_maps = []
    for core in range(N_CORES):
        b, g = divmod(core, 2)
        xr = np.roll(x[b], -TQ * g, axis=0)
        in_maps.append({'xq': np.ascontiguousarray(xr[:TQ] + resid),
                        'xT': pair4(xr.T),
                        **shared})
    return in_maps


_NC = None
_NC_GB = None


def kernel(**inputs):
    global _NC, _NC_GB
    apply_gb = not (np.all(np.asarray(inputs['gamma']) == 1.0)
                    and np.all(np.asarray(inputs['beta']) == 0.0))
    if _NC is None or _NC_GB != apply_gb:
        _NC = build(apply_gb)
        _NC_GB = apply_gb
    in_maps = make_in_maps(inputs)
    res = bass_utils.run_bass_kernel_spmd(_NC, in_maps,
                                          core_ids=list(range(N_CORES)))
    out = np.empty((B, T, D), dtype=np.float32)
    for core in range(N_CORES):
        b, g = divmod(core, 2)
        out[b, TQ * g:TQ * (g + 1)] = res.results[core]['y']
    return out


# revision 68
# speedup vs baseline: 1.0057x; 1.0057x over previous
"""Multi-head attention block (QKV proj + softmax attention + out-proj +
residual + LayerNorm) on 8 TRN2 NeuronCores.  ~290us (1.6x over the
460us AllGather/bf16 baseline).

Sharding: core = (batch b, token-half g). Each core computes K/V for ALL 8
heads over the full 2048 tokens of its batch (replicated across the pair —
no collectives; the redundant projections are cheap in fp8 DoubleRow and
the PE slack absorbs them), attention for its 1024 query tokens, then the
output projection, residual and LayerNorm for its token half. Outputs are
disjoint [1024, 1024] shards concatenated on the host.

Inputs are token-rotated per core on the host so rows 0..1023 of the
per-core x are that core's query tokens (softmax over k is permutation-
invariant). The host pre-arranges x^T and all weights in the fp8
DoubleRow pair layout [sc][p, i, n] (contraction index = sc*256+i*128+p)
so every DMA is contiguous 2-4KB rows per partition.

Precision/engine strategy:
- All GEMMs with a >=256-deep contraction (Q/K/V projections, PV, the
  softmax-denominator "ones" matmuls, out-projection) run fp8e4
  DoubleRow (2 contraction rows per PE cell -> 2x MACs/cycle).
  Attention scores stay bf16 (contraction is only 128 = no pairing).
- softmax has no max-subtraction (scores/sqrt(128) ~ N(0, 0.33), exp is
  tiny); weights pt go straight to fp8.  exp is split across TWO
  engines: ScalarE table Exp (fp8 out) for 5/8 of k-chunks, and a DVE
  "Schraudolph" exp for 3/8 — e4m3 bits of exp(y) ~= uint8(y*8*log2e +
  56.5), one tensor_scalar, with PWL error ~ e4m3 quantization and
  under/overflow >9 sigma away.
- K bias dropped exactly (constant over k per query -> softmax
  invariant); V bias and out-proj bias folded on the host into the
  residual via bv @ Wo; the fp32 residual + LayerNorm damp the ~3-5%
  fp8 attention noise to ~7e-4 output error.

Schedule: two query passes of 512 (psum: 3 scores + 2 proj + 1 ctx +
1 sum + 1 y bank). Per (pass, head), 8 super-chunks of 256 k-tokens run
a software pipeline (PV lags scores/exp by one super-chunk); projection
tasks drip-feed into the PE between super-chunks with issue-order
deadlines; pass 1 (exp-bound) absorbs the second-half Q projections and
the out-projections of q-half 0. LayerNorm finishes run in the tail so
the ScalarE Sqrt never thrashes the Exp activation table.
"""

import contextlib
import sys

if '/opt/trn_rl_repo' not in sys.path:
    sys.path.insert(0, '/opt/trn_rl_repo')

import ml_dtypes
import numpy as np

import concourse.bacc as bacc
import concourse.bass as bass
import concourse.bass_utils as bass_utils
import concourse.tile as tile
from concourse import mybir

B, T, D, H = 4, 2048, 1024, 8
DH = 128            # head dim
TQ = T // 2         # query tokens per core
N_CORES = 8
KC = T // 128       # k-token chunks of 128
SC = T // 256       # k-token super-chunks of 256 (DoubleRow pairs) -> 8
NSC = D // 256      # contraction super-chunks for d=1024 -> 4
QC = TQ // 128      # q-token chunks of 128
EPS = 1e-5
ISCALE = 1.0 / float(np.sqrt(DH))
F32 = mybir.dt.float32
BF16 = mybir.dt.bfloat16
FP8 = mybir.dt.float8e4
AF = mybir.ActivationFunctionType
ALU = mybir.AluOpType
DR = mybir.MatmulPerfMode.DoubleRow
BF = ml_dtypes.bfloat16
F8 = ml_dtypes.float8_e4m3fn
# Schraudolph fp8 exp: e4m3 bits of exp(s*ISCALE) ~= s*SCH_A + SCH_B
SCH_A = float(8.0 * np.log2(np.e) * ISCALE)
SCH_B = 56.5


def _body(nc, tc, ap, es, apply_gb):
    xq, xT, Wq, bq, Wk, Wv, Wo, gamma, beta, y = (
        ap['xq'], ap['xT'], ap['Wq'], ap['bq'], ap['Wk'],
        ap['Wv'], ap['Wo'], ap['gamma'], ap['beta'], ap['y'])

    consts = es.enter_context(tc.tile_pool(name="consts", bufs=1))
    xt_pool = es.enter_context(tc.tile_pool(name="xt", bufs=1))
    q_pool = es.enter_context(tc.tile_pool(name="q", bufs=1))
    k_pool = es.enter_context(tc.tile_pool(name="k", bufs=1))
    v_pool = es.enter_context(tc.tile_pool(name="v", bufs=1))
    ctx_pool = es.enter_context(tc.tile_pool(name="ctx", bufs=1))
    wo_pool = es.enter_context(tc.tile_pool(name="wo", bufs=1))
    y1_pool = es.enter_context(tc.tile_pool(name="y1st", bufs=1))
    pt_pool = es.enter_context(tc.tile_pool(name="pt", bufs=2))
    sums_pool = es.enter_context(tc.tile_pool(name="sums", bufs=2))
    xr_pool = es.enter_context(tc.tile_pool(name="xr", bufs=2))
    y3_pool = es.enter_context(tc.tile_pool(name="y3", bufs=2))
    ln_pool = es.enter_context(tc.tile_pool(name="ln", bufs=4))

    # ---- constants -------------------------------------------------------
    # DR ldweights needs the pair-dim stride % 16 == 0, so pad the free dim
    ones2_t = consts.tile([128, 2, 16], FP8, tag="ones2")
    nc.vector.memset(ones2_t, 1.0)
    ones2 = ones2_t[:, :, 0:1]
    eps_t = consts.tile([128, 1], F32, tag="eps")
    nc.vector.memset(eps_t, EPS)

    # partition-broadcast rows (per-feature vectors used on the free dim)
    def bcast128(name, src):
        t = consts.tile([128, D], F32, tag=name, name=name)
        src_b = bass.AP(tensor=src.tensor, offset=src.offset,
                        ap=[[0, 128]] + src.ap)
        nc.sync.dma_start(out=t, in_=src_b)
        return t

    gb = [bcast128("gamma_b", gamma), bcast128("beta_b", beta)] \
        if apply_gb else None

    # per-head bias layout: bias_t[p, h] = b[h*128 + p]
    bq_t = consts.tile([128, H], F32, tag="bq")
    nc.sync.dma_start(out=bq_t, in_=bq.rearrange("(h p) -> p h", p=128))

    # x^T in DoubleRow pair layout: xTp[sc][p, i, t] = xT[sc*256+i*128+p, t]
    # and all weights SBUF-resident in pair layout. DMAs are interleaved
    # sc-wise so the first projection matmuls can start ~3us in instead of
    # waiting for whole tensors.
    # x^T pair tiles split into two 1024-column halves so the first
    # projection tasks only wait on half the bytes
    xtA = [xt_pool.tile([128, 2, TQ], FP8, tag=f"xta{sc}", name=f"xta{sc}")
           for sc in range(NSC)]
    xtB = [xt_pool.tile([128, 2, TQ], FP8, tag=f"xtb{sc}", name=f"xtb{sc}")
           for sc in range(NSC)]
    wq_sb = [wo_pool.tile([128, 2, D], FP8, tag=f"wq{sc}", name=f"wq{sc}")
             for sc in range(NSC)]
    wk_sb = [wo_pool.tile([128, 2, D], FP8, tag=f"wk{sc}", name=f"wk{sc}")
             for sc in range(NSC)]
    wv_sb = [wo_pool.tile([128, 2, D], FP8, tag=f"wv{sc}", name=f"wv{sc}")
             for sc in range(NSC)]

    # Startup-critical set first: k_task(0,0) needs xtA+wk (sync queue),
    # q_task(0,0) needs wq (scalar queue, in parallel); the rest stream
    # behind. The host pre-arranges the pair layout, so these are
    # contiguous row DMAs.
    for sc in range(NSC):
        nc.sync.dma_start(out=xtA[sc], in_=xT[sc][:, :, 0:TQ])
        nc.sync.dma_start(out=wk_sb[sc], in_=Wk[sc])
        nc.scalar.dma_start(out=wq_sb[sc], in_=Wq[sc])
    for sc in range(NSC):
        nc.scalar.dma_start(out=wv_sb[sc], in_=Wv[sc])
        nc.sync.dma_start(out=xtB[sc], in_=xT[sc][:, :, TQ:T])

    # persistent per-head tensors
    KT = [k_pool.tile([128, T], BF16, tag=f"k{h}", name=f"k{h}")
          for h in range(H)]
    QT = [q_pool.tile([128, TQ], BF16, tag=f"q{h}", name=f"q{h}")
          for h in range(H)]
    VT = v_pool.tile([128, H, KC, 128], FP8, tag="vt", name="vt")
    ctx = ctx_pool.tile([128, H, TQ], FP8, tag="ctx")

    # Wo blocks in pair layout (DMA'd mid-attention, used in phase 3)
    wo_sb = [wo_pool.tile([128, 2, D], FP8, tag=f"wo{sc}", name=f"wo{sc}")
             for sc in range(NSC)]

    with contextlib.ExitStack() as es2:
        proj_ps = es2.enter_context(tc.tile_pool(name="proj_ps", bufs=2,
                                                 space="PSUM"))
        scores_ps = es2.enter_context(tc.tile_pool(name="scores_ps", bufs=3,
                                                   space="PSUM"))
        ctx_psum = es2.enter_context(tc.tile_pool(name="ctx_ps", bufs=1,
                                                  space="PSUM"))
        sum_psum = es2.enter_context(tc.tile_pool(name="sum_ps", bufs=1,
                                                  space="PSUM"))
        y_psum = es2.enter_context(tc.tile_pool(name="y_ps", bufs=1,
                                                space="PSUM"))

        # ---- projection tasks (each: 4 DR matmuls + psum-freeing copy) ----
        def q_task(h, nt):
            nsl = slice(nt * 512, (nt + 1) * 512)
            hsl = slice(h * 128, (h + 1) * 128)
            pp = proj_ps.tile([128, 512], F32, tag="pp", name="pp")
            for sc in range(NSC):
                nc.tensor.matmul(pp, wq_sb[sc][:, :, hsl],
                                 xtA[sc][:, :, nsl], perf_mode=DR,
                                 start=(sc == 0), stop=(sc == NSC - 1))
            nc.scalar.add(out=QT[h][:, nsl], in_=pp, add=bq_t[:, h:h + 1])

        def k_task(h, nt):
            nsl = slice(nt * 512, (nt + 1) * 512)
            xt_h = xtA if nt < 2 else xtB
            xsl = slice((nt % 2) * 512, (nt % 2 + 1) * 512)
            hsl = slice(h * 128, (h + 1) * 128)
            pp = proj_ps.tile([128, 512], F32, tag="pp", name="pp")
            for sc in range(NSC):
                nc.tensor.matmul(pp, wk_sb[sc][:, :, hsl],
                                 xt_h[sc][:, :, xsl], perf_mode=DR,
                                 start=(sc == 0), stop=(sc == NSC - 1))
            nc.scalar.copy(out=KT[h][:, nsl], in_=pp)

        def v_task(blk, fh):
            """V token-major: x-stationary. out psum [128 tok, 512 feats]
            = heads fh*4..fh*4+3; one strided copy into VT."""
            xt_h = xtA if blk < 8 else xtB
            tsl = slice((blk % 8) * 128, (blk % 8 + 1) * 128)
            fsl = slice(fh * 512, (fh + 1) * 512)
            pp = proj_ps.tile([128, 512], F32, tag="pp", name="pp")
            for sc in range(NSC):
                nc.tensor.matmul(pp, xt_h[sc][:, :, tsl],
                                 wv_sb[sc][:, :, fsl], perf_mode=DR,
                                 start=(sc == 0), stop=(sc == NSC - 1))
            nc.vector.tensor_copy(
                out=VT[:, fh * 4:(fh + 1) * 4, blk, :],
                in_=pp.rearrange("p (h m) -> p h m", h=4))

        # ---- out-projection + residual for one 128-q block; the LN finish
        # of the 4 background blocks is deferred to the tail so the ScalarE
        # Sqrt never interleaves with Exp (activation-table thrash)
        y1_st = [y1_pool.tile([128, D], F32, tag=f"y1s{qc}", name=f"y1s{qc}")
                 for qc in range(4)]
        mv_st = ln_pool.tile([128, QC, 2], F32, tag="mv_st")

        def op_mm(qc, y1):
            qs = slice(qc * 128, (qc + 1) * 128)
            xr = xr_pool.tile([128, D], F32, tag="xr")
            nc.sync.dma_start(out=xr, in_=xq[qc * 128:(qc + 1) * 128, :])
            stats = ln_pool.tile([128, 2, 6], F32, tag="stats")
            y1g = y1.rearrange("p (n f) -> p n f", f=512)
            for no in range(D // 512):
                nsl = slice(no * 512, (no + 1) * 512)
                y_ps = y_psum.tile([128, 512], F32, tag="y_ps")
                for sc in range(NSC):
                    nc.tensor.matmul(y_ps,
                                     ctx[:, 2 * sc:2 * sc + 2, qs],
                                     wo_sb[sc][:, :, nsl], perf_mode=DR,
                                     start=(sc == 0), stop=(sc == NSC - 1))
                nc.vector.tensor_add(out=y1[:, nsl], in0=y_ps,
                                     in1=xr[:, nsl])  # resid (+bo)
                nc.vector.bn_stats(out=stats[:, no, :], in_=y1g[:, no, :])
            nc.vector.bn_aggr(out=mv_st[:, qc, :], in_=stats)

        def ln_finish(qc, y1):
            qs = slice(qc * 128, (qc + 1) * 128)
            std = ln_pool.tile([128, 1], F32, tag="std")
            nc.scalar.activation(out=std, in_=mv_st[:, qc, 1:2],
                                 func=AF.Sqrt, bias=eps_t)
            rstd = ln_pool.tile([128, 1], F32, tag="rstd")
            nc.vector.reciprocal(out=rstd, in_=std)
            # y2 = (y1 - mu)*rstd split across DVE and ScalarE
            # (scalar form: y1*rstd + (-mu*rstd))
            nmr = ln_pool.tile([128, 1], F32, tag="nmr")
            nc.vector.tensor_scalar(out=nmr, in0=rstd,
                                    scalar1=mv_st[:, qc, 0:1],
                                    scalar2=-1.0,
                                    op0=ALU.mult, op1=ALU.mult)
            y2 = y3_pool.tile([128, D], F32, tag="y2")
            nc.vector.tensor_scalar(out=y2[:, 0:512], in0=y1[:, 0:512],
                                    scalar1=mv_st[:, qc, 0:1],
                                    scalar2=rstd, op0=ALU.subtract,
                                    op1=ALU.mult)
            nc.scalar.activation(out=y2[:, 512:D], in_=y1[:, 512:D],
                                 func=AF.Identity, scale=rstd, bias=nmr)
            if apply_gb:
                nc.vector.tensor_mul(out=y2, in0=y2, in1=gb[0])
                nc.vector.tensor_add(out=y2, in0=y2, in1=gb[1])
            nc.sync.dma_start(out=y[qs, :], in_=y2)

        # ---- background task schedule (drip-fed into the attention loop).
        # ISSUE-ORDER deadlines (Tile derives deps from program order, so a
        # consumer must be issued after its producer): pass qp / head h
        # scores use K[h] (nt cols) and Q[h][:, qp*512:]; pv(s) (issued in
        # super s) reads VT[.][:, 2s:2s+2, :]; out_proj(qc<4) runs in pass 1
        # and reads every head's pass-0 ctx.
        K = lambda h, nt: (lambda: k_task(h, nt))
        Q = lambda h, nt: (lambda: q_task(h, nt))
        V = lambda b, fh: (lambda: v_task(b, fh))
        OP = lambda qc: (lambda: op_mm(qc, y1_st[qc]))
        bg_step = {
            # pass 0 head 0 pops 3/super; VA pair (2s+2, 2s+3) lands one
            # super ahead of pv(s+1)'s issue; K(0,nt) before super 2nt-1.
            (0, 0): [V(2, 0), V(3, 0), K(0, 1),
                     V(4, 0), V(5, 0), K(0, 2),
                     V(6, 0), V(7, 0), K(0, 3),
                     V(8, 0), V(9, 0), K(1, 0),
                     V(10, 0), V(11, 0), K(1, 1),
                     V(12, 0), V(13, 0), K(1, 2),
                     V(14, 0), V(15, 0), K(1, 3),
                     Q(1, 0), K(2, 0), K(2, 1)],
            (0, 1): [K(2, 2), K(2, 3), Q(2, 0)]
                    + [V(b, 1) for b in range(10)]
                    + [K(3, 0), K(3, 1), K(3, 2)],
            (0, 2): [K(3, 3), Q(3, 0)]
                    + [V(b, 1) for b in range(10, 16)]
                    + [K(4, 0), K(4, 1), K(4, 2), K(4, 3), Q(4, 0),
                       K(5, 0), K(5, 1), K(5, 2)],
            (0, 3): [K(5, 3), Q(5, 0),
                     K(6, 0), K(6, 1), K(6, 2), K(6, 3), Q(6, 0),
                     K(7, 0), K(7, 1), K(7, 2), K(7, 3), Q(7, 0),
                     Q(0, 1)],
            # pass 1 is exp-bound with an idle PE: fill it with the
            # deferred second-half Q projections and the out-projections
            # of the already-complete q-half 0
            (1, 0): [Q(1, 1), OP(0)], (1, 1): [Q(2, 1), OP(1)],
            (1, 2): [Q(3, 1), OP(2)], (1, 3): [Q(4, 1), OP(3)],
            (1, 4): [Q(5, 1)], (1, 5): [Q(6, 1)], (1, 6): [Q(7, 1)],
        }
        bg = []
        bg_i = 0

        def pops(n):
            nonlocal bg_i
            for _ in range(n):
                if bg_i < len(bg):
                    bg[bg_i]()
                    bg_i += 1

        # ---- prologue: just enough for pass 0 head 0 to start ------------
        k_task(0, 0)
        q_task(0, 0)
        v_task(0, 0)
        v_task(1, 0)

        # ---- attention: two query passes of 512 --------------------------
        for qp in range(2):
            qsl = slice(qp * 512, (qp + 1) * 512)
            for h in range(H):
                bg += bg_step.get((qp, h), [])
                pop_rate = 3 if (qp, h) == (0, 0) else (2 if qp == 0 else 1)
                if (qp, h) == (0, 2):
                    for sc in range(NSC):
                        nc.sync.dma_start(out=wo_sb[sc], in_=Wo[sc])

                ctx_ps = ctx_psum.tile([128, 512], F32, tag="ctx_ps")
                sum_ps = sum_psum.tile([1, 512], F32, tag="sum_ps")

                def scores_exp(s):
                    pt = pt_pool.tile([128, 2, 512], FP8, tag="pt", name="pt")
                    for i in range(2):
                        kc = 2 * s + i
                        ks = slice(kc * 128, (kc + 1) * 128)
                        sp = scores_ps.tile([128, 512], F32, tag="sp",
                                            name="sp")
                        nc.tensor.matmul(sp, KT[h][:, ks], QT[h][:, qsl],
                                         start=True, stop=True)
                        if kc % 8 in (1, 3, 5):
                            # DVE "Schraudolph" exp: e4m3 bits of exp(y)
                            # ~= uint8(y*8*log2e + 56.5) (PWL between
                            # powers of 2, error ~ e4m3 quantization)
                            nc.vector.tensor_scalar(
                                out=pt[:, i, :].bitcast(mybir.dt.uint8),
                                in0=sp, scalar1=SCH_A, scalar2=SCH_B,
                                op0=ALU.mult, op1=ALU.add)
                        else:
                            nc.scalar.activation(out=pt[:, i, :], in_=sp,
                                                 func=AF.Exp, scale=ISCALE)
                    return pt

                def pv(s, pt):
                    nc.tensor.matmul(ctx_ps, VT[:, h, 2 * s:2 * s + 2, :],
                                     pt, perf_mode=DR,
                                     start=(s == 0), stop=(s == SC - 1))
                    nc.tensor.matmul(sum_ps, ones2, pt, perf_mode=DR,
                                     start=(s == 0), stop=(s == SC - 1))

                # software pipeline: PV lags scores/exp by one super-chunk
                pt_cur = scores_exp(0)
                for s in range(SC):
                    pt_next = scores_exp(s + 1) if s + 1 < SC else None
                    pv(s, pt_cur)
                    pops(pop_rate)
                    pt_cur = pt_next

                # normalize: ctx[h, qsl] = ctx_ps / sum + bv. The psum
                # tiles are freed by quick copies so the next head's PV /
                # ones chains (same single-buffered banks) don't wait on
                # the full reciprocal+broadcast chain.
                # (bv is folded into xq on the host via bv @ Wo)
                ctxs = sums_pool.tile([128, 512], F32, tag="ctxs")
                nc.vector.tensor_copy(out=ctxs, in_=ctx_ps)
                rsum = sums_pool.tile([1, 512], F32, tag="rsum")
                nc.vector.reciprocal_approx_fast(out=rsum, in_=sum_ps)
                rsum_b = sums_pool.tile([128, 512], F32, tag="rsum_b")
                nc.gpsimd.partition_broadcast(rsum_b, rsum, channels=128)
                nc.vector.tensor_mul(out=ctx[:, h, qsl], in0=ctxs,
                                     in1=rsum_b)

        # ---- tail: remaining out-projection blocks with LN finishes
        # interleaved (all exps are done: one Exp->Sqrt table swap); the
        # deferred finishes of the background blocks fill the gaps
        y1_t = [y3_pool.tile([128, D], F32, tag=f"y1{qc % 2}", name="y1")
                for qc in range(4, QC)]
        for qc in range(4, QC):
            op_mm(qc, y1_t[qc - 4])
            if qc >= 5:
                ln_finish(qc - 5, y1_st[qc - 5])
        ln_finish(3, y1_st[3])
        for qc in range(4, QC):
            ln_finish(qc, y1_t[qc - 4])


def build(apply_gb=True):
    nc = bacc.Bacc("TRN2", target_bir_lowering=False, debug=False,
                   enable_asserts=False, num_devices=N_CORES)
    ap = {}
    ap['xq'] = nc.dram_tensor("xq", [TQ, D], F32, kind="ExternalInput").ap()
    ap['xT'] = nc.dram_tensor("xT", [NSC, 128, 2, T], FP8,
                              kind="ExternalInput").ap()
    ap['bq'] = nc.dram_tensor("bq", [D], F32, kind="ExternalInput").ap()
    for w in ('Wq', 'Wk', 'Wv', 'Wo'):
        ap[w] = nc.dram_tensor(w, [NSC, 128, 2, D], FP8,
                               kind="ExternalInput").ap()
    ap['gamma'] = nc.dram_tensor("gamma", [D], F32, kind="ExternalInput").ap()
    ap['beta'] = nc.dram_tensor("beta", [D], F32, kind="ExternalInput").ap()
    ap['y'] = nc.dram_tensor("y", [TQ, D], F32, kind="ExternalOutput").ap()

    with tile.TileContext(nc) as tc, contextlib.ExitStack() as es:
        _body(nc, tc, ap, es, apply_gb)
    nc.compile()
    return nc


def make_in_maps(inputs):
    """Per-core input maps; x token-rotated so q tokens come first."""
    f32 = {k: np.ascontiguousarray(np.asarray(v, dtype=np.float32))
           for k, v in inputs.items()}
    shared = {k: f32[k] for k in ('bq', 'gamma', 'beta')}

    def pair4(a):  # [D, N] -> [NSC, 128, 2, N] DoubleRow pair layout
        return np.ascontiguousarray(
            a.astype(F8).reshape(NSC, 2, 128, a.shape[1]).transpose(
                0, 2, 1, 3))

    for w in ('Wq', 'Wk', 'Wv', 'Wo'):
        shared[w] = pair4(f32[w])
    x = f32['x']
    # fold both the out-proj bias and the V bias into the residual:
    # (ctx0 + bv) @ Wo + bo + x = ctx0 @ Wo + (x + bo + bv@Wo)
    resid = f32['bo'] + f32['bv'] @ f32['Wo']
    in_maps = []
    for core in range(N_CORES):
        b, g = divmod(core, 2)
        xr = np.roll(x[b], -TQ * g, axis=0)
        in_maps.append({'xq': np.ascontiguousarray(xr[:TQ] + resid),
                        'xT': pair4(xr.T),
                        **shared})
    return in_maps


_NC = None
_NC_GB = None


def kernel(**inputs):
    global _NC, _NC_GB
    apply_gb = not (np.all(np.asarray(inputs['gamma']) == 1.0)
                    and np.all(np.asarray(inputs['beta']) == 0.0))
    if _NC is None or _NC_GB != apply_gb:
        _NC = build(apply_gb)
        _NC_GB = apply_gb
    in_maps = make_in_maps(inputs)
    res = bass_utils.run_bass_kernel_spmd(_NC, in_maps,
                                          core_ids=list(range(N_CORES)))
    out = np.empty((B, T, D), dtype=np.float32)
    for core in range(N_CORES):
        b, g = divmod(core, 2)
        out[b, TQ * g:TQ * (g + 1)] = res.results[core]['y']
    return out


# revision 70
# speedup vs baseline: 1.0109x; 1.0051x over previous
"""Multi-head attention block (QKV proj + softmax attention + out-proj +
residual + LayerNorm) on 8 TRN2 NeuronCores.  ~290us (1.6x over the
460us AllGather/bf16 baseline).

Sharding: core = (batch b, token-half g). Each core computes K/V for ALL 8
heads over the full 2048 tokens of its batch (replicated across the pair —
no collectives; the redundant projections are cheap in fp8 DoubleRow and
the PE slack absorbs them), attention for its 1024 query tokens, then the
output projection, residual and LayerNorm for its token half. Outputs are
disjoint [1024, 1024] shards concatenated on the host.

Inputs are token-rotated per core on the host so rows 0..1023 of the
per-core x are that core's query tokens (softmax over k is permutation-
invariant). The host pre-arranges x^T and all weights in the fp8
DoubleRow pair layout [sc][p, i, n] (contraction index = sc*256+i*128+p)
so every DMA is contiguous 2-4KB rows per partition.

Precision/engine strategy:
- All GEMMs with a >=256-deep contraction (Q/K/V projections, PV, the
  softmax-denominator "ones" matmuls, out-projection) run fp8e4
  DoubleRow (2 contraction rows per PE cell -> 2x MACs/cycle).
  Attention scores stay bf16 (contraction is only 128 = no pairing).
- softmax has no max-subtraction (scores/sqrt(128) ~ N(0, 0.33), exp is
  tiny); weights pt go straight to fp8.  exp is split across TWO
  engines: ScalarE table Exp (fp8 out) for 5/8 of k-chunks, and a DVE
  "Schraudolph" exp for 3/8 — e4m3 bits of exp(y) ~= uint8(y*8*log2e +
  56.5), one tensor_scalar, with PWL error ~ e4m3 quantization and
  under/overflow >9 sigma away.
- K bias dropped exactly (constant over k per query -> softmax
  invariant); V bias and out-proj bias folded on the host into the
  residual via bv @ Wo; the fp32 residual + LayerNorm damp the ~3-5%
  fp8 attention noise to ~7e-4 output error.

Schedule: two query passes of 512 (psum: 3 scores + 2 proj + 1 ctx +
1 sum + 1 y bank). Per (pass, head), 8 super-chunks of 256 k-tokens run
a software pipeline (PV lags scores/exp by one super-chunk); projection
tasks drip-feed into the PE between super-chunks with issue-order
deadlines; pass 1 (exp-bound) absorbs the second-half Q projections and
the out-projections of q-half 0. LayerNorm finishes run in the tail so
the ScalarE Sqrt never thrashes the Exp activation table.
"""

import contextlib
import sys

if '/opt/trn_rl_repo' not in sys.path:
    sys.path.insert(0, '/opt/trn_rl_repo')

import ml_dtypes
import numpy as np

import concourse.bacc as bacc
import concourse.bass as bass
import concourse.bass_utils as bass_utils
import concourse.tile as tile
from concourse import mybir

B, T, D, H = 4, 2048, 1024, 8
DH = 128            # head dim
TQ = T // 2         # query tokens per core
N_CORES = 8
KC = T // 128       # k-token chunks of 128
SC = T // 256       # k-token super-chunks of 256 (DoubleRow pairs) -> 8
NSC = D // 256      # contraction super-chunks for d=1024 -> 4
QC = TQ // 128      # q-token chunks of 128
EPS = 1e-5
ISCALE = 1.0 / float(np.sqrt(DH))
F32 = mybir.dt.float32
BF16 = mybir.dt.bfloat16
FP8 = mybir.dt.float8e4
AF = mybir.ActivationFunctionType
ALU = mybir.AluOpType
DR = mybir.MatmulPerfMode.DoubleRow
BF = ml_dtypes.bfloat16
F8 = ml_dtypes.float8_e4m3fn
# Schraudolph fp8 exp: e4m3 bits of exp(s*ISCALE) ~= s*SCH_A + SCH_B
SCH_A = float(8.0 * np.log2(np.e) * ISCALE)
SCH_B = 56.5


def _body(nc, tc, ap, es, apply_gb):
    xq, xT, Wq, bq, Wk, Wv, Wo, gamma, beta, y = (
        ap['xq'], ap['xT'], ap['Wq'], ap['bq'], ap['Wk'],
        ap['Wv'], ap['Wo'], ap['gamma'], ap['beta'], ap['y'])

    consts = es.enter_context(tc.tile_pool(name="consts", bufs=1))
    xt_pool = es.enter_context(tc.tile_pool(name="xt", bufs=1))
    q_pool = es.enter_context(tc.tile_pool(name="q", bufs=1))
    k_pool = es.enter_context(tc.tile_pool(name="k", bufs=1))
    v_pool = es.enter_context(tc.tile_pool(name="v", bufs=1))
    ctx_pool = es.enter_context(tc.tile_pool(name="ctx", bufs=1))
    wo_pool = es.enter_context(tc.tile_pool(name="wo", bufs=1))
    y1_pool = es.enter_context(tc.tile_pool(name="y1st", bufs=1))
    pt_pool = es.enter_context(tc.tile_pool(name="pt", bufs=3))
    sums_pool = es.enter_context(tc.tile_pool(name="sums", bufs=3))
    xr_pool = es.enter_context(tc.tile_pool(name="xr", bufs=2))
    y3_pool = es.enter_context(tc.tile_pool(name="y3", bufs=2))
    ln_pool = es.enter_context(tc.tile_pool(name="ln", bufs=4))

    # ---- constants -------------------------------------------------------
    # DR ldweights needs the pair-dim stride % 16 == 0, so pad the free dim
    ones2_t = consts.tile([128, 2, 16], FP8, tag="ones2")
    nc.vector.memset(ones2_t, 1.0)
    ones2 = ones2_t[:, :, 0:1]
    eps_t = consts.tile([128, 1], F32, tag="eps")
    nc.vector.memset(eps_t, EPS)

    # partition-broadcast rows (per-feature vectors used on the free dim)
    def bcast128(name, src):
        t = consts.tile([128, D], F32, tag=name, name=name)
        src_b = bass.AP(tensor=src.tensor, offset=src.offset,
                        ap=[[0, 128]] + src.ap)
        nc.sync.dma_start(out=t, in_=src_b)
        return t

    gb = [bcast128("gamma_b", gamma), bcast128("beta_b", beta)] \
        if apply_gb else None

    # per-head bias layout: bias_t[p, h] = b[h*128 + p]
    bq_t = consts.tile([128, H], F32, tag="bq")
    nc.sync.dma_start(out=bq_t, in_=bq.rearrange("(h p) -> p h", p=128))

    # x^T in DoubleRow pair layout: xTp[sc][p, i, t] = xT[sc*256+i*128+p, t]
    # and all weights SBUF-resident in pair layout. DMAs are interleaved
    # sc-wise so the first projection matmuls can start ~3us in instead of
    # waiting for whole tensors.
    # x^T pair tiles split into two 1024-column halves so the first
    # projection tasks only wait on half the bytes
    xtA = [xt_pool.tile([128, 2, TQ], FP8, tag=f"xta{sc}", name=f"xta{sc}")
           for sc in range(NSC)]
    xtB = [xt_pool.tile([128, 2, TQ], FP8, tag=f"xtb{sc}", name=f"xtb{sc}")
           for sc in range(NSC)]
    wq_sb = [wo_pool.tile([128, 2, D], FP8, tag=f"wq{sc}", name=f"wq{sc}")
             for sc in range(NSC)]
    wk_sb = [wo_pool.tile([128, 2, D], FP8, tag=f"wk{sc}", name=f"wk{sc}")
             for sc in range(NSC)]
    wv_sb = [wo_pool.tile([128, 2, D], FP8, tag=f"wv{sc}", name=f"wv{sc}")
             for sc in range(NSC)]

    # Startup-critical set first, split across all three DMA trigger
    # queues (sync + scalar HWDGE + gpsimd SWDGE) — a single queue issues
    # in order at ~0.6us/DMA and serializes on its semaphore ring, which
    # otherwise delays the first matmul to ~21us. The host pre-arranges
    # the pair layout, so these are contiguous row DMAs.
    for sc in range(0, NSC, 2):
        nc.sync.dma_start(out=xtA[sc], in_=xT[sc][:, :, 0:TQ])
        nc.sync.dma_start(out=wk_sb[sc], in_=Wk[sc])
        nc.scalar.dma_start(out=xtA[sc + 1], in_=xT[sc + 1][:, :, 0:TQ])
        nc.scalar.dma_start(out=wk_sb[sc + 1], in_=Wk[sc + 1])
    for sc in range(NSC):
        nc.gpsimd.dma_start(out=wq_sb[sc], in_=Wq[sc])
    for sc in range(NSC):
        nc.scalar.dma_start(out=wv_sb[sc], in_=Wv[sc])
        nc.sync.dma_start(out=xtB[sc], in_=xT[sc][:, :, TQ:T])

    # persistent per-head tensors
    KT = [k_pool.tile([128, T], BF16, tag=f"k{h}", name=f"k{h}")
          for h in range(H)]
    QT = [q_pool.tile([128, TQ], BF16, tag=f"q{h}", name=f"q{h}")
          for h in range(H)]
    VT = v_pool.tile([128, H, KC, 128], FP8, tag="vt", name="vt")
    ctx = ctx_pool.tile([128, H, TQ], FP8, tag="ctx")

    # Wo blocks in pair layout (DMA'd mid-attention, used in phase 3)
    wo_sb = [wo_pool.tile([128, 2, D], FP8, tag=f"wo{sc}", name=f"wo{sc}")
             for sc in range(NSC)]

    with contextlib.ExitStack() as es2:
        proj_ps = es2.enter_context(tc.tile_pool(name="proj_ps", bufs=2,
                                                 space="PSUM"))
        scores_ps = es2.enter_context(tc.tile_pool(name="scores_ps", bufs=3,
                                                   space="PSUM"))
        ctx_psum = es2.enter_context(tc.tile_pool(name="ctx_ps", bufs=1,
                                                  space="PSUM"))
        sum_psum = es2.enter_context(tc.tile_pool(name="sum_ps", bufs=1,
                                                  space="PSUM"))
        y_psum = es2.enter_context(tc.tile_pool(name="y_ps", bufs=1,
                                                space="PSUM"))

        # ---- projection tasks (each: 4 DR matmuls + psum-freeing copy) ----
        def q_task(h, nt):
            nsl = slice(nt * 512, (nt + 1) * 512)
            hsl = slice(h * 128, (h + 1) * 128)
            pp = proj_ps.tile([128, 512], F32, tag="pp", name="pp")
            for sc in range(NSC):
                nc.tensor.matmul(pp, wq_sb[sc][:, :, hsl],
                                 xtA[sc][:, :, nsl], perf_mode=DR,
                                 start=(sc == 0), stop=(sc == NSC - 1))
            nc.scalar.add(out=QT[h][:, nsl], in_=pp, add=bq_t[:, h:h + 1])

        def k_task(h, nt):
            nsl = slice(nt * 512, (nt + 1) * 512)
            xt_h = xtA if nt < 2 else xtB
            xsl = slice((nt % 2) * 512, (nt % 2 + 1) * 512)
            hsl = slice(h * 128, (h + 1) * 128)
            pp = proj_ps.tile([128, 512], F32, tag="pp", name="pp")
            for sc in range(NSC):
                nc.tensor.matmul(pp, wk_sb[sc][:, :, hsl],
                                 xt_h[sc][:, :, xsl], perf_mode=DR,
                                 start=(sc == 0), stop=(sc == NSC - 1))
            nc.scalar.copy(out=KT[h][:, nsl], in_=pp)

        def v_task(blk, fh):
            """V token-major: x-stationary. out psum [128 tok, 512 feats]
            = heads fh*4..fh*4+3; one strided copy into VT."""
            xt_h = xtA if blk < 8 else xtB
            tsl = slice((blk % 8) * 128, (blk % 8 + 1) * 128)
            fsl = slice(fh * 512, (fh + 1) * 512)
            pp = proj_ps.tile([128, 512], F32, tag="pp", name="pp")
            for sc in range(NSC):
                nc.tensor.matmul(pp, xt_h[sc][:, :, tsl],
                                 wv_sb[sc][:, :, fsl], perf_mode=DR,
                                 start=(sc == 0), stop=(sc == NSC - 1))
            nc.vector.tensor_copy(
                out=VT[:, fh * 4:(fh + 1) * 4, blk, :],
                in_=pp.rearrange("p (h m) -> p h m", h=4))

        # ---- out-projection + residual for one 128-q block; the LN finish
        # of the 4 background blocks is deferred to the tail so the ScalarE
        # Sqrt never interleaves with Exp (activation-table thrash)
        y1_st = [y1_pool.tile([128, D], F32, tag=f"y1s{qc}", name=f"y1s{qc}")
                 for qc in range(4)]
        mv_st = ln_pool.tile([128, QC, 2], F32, tag="mv_st")

        def op_mm(qc, y1):
            qs = slice(qc * 128, (qc + 1) * 128)
            xr = xr_pool.tile([128, D], F32, tag="xr")
            nc.sync.dma_start(out=xr, in_=xq[qc * 128:(qc + 1) * 128, :])
            stats = ln_pool.tile([128, 2, 6], F32, tag="stats")
            y1g = y1.rearrange("p (n f) -> p n f", f=512)
            for no in range(D // 512):
                nsl = slice(no * 512, (no + 1) * 512)
                y_ps = y_psum.tile([128, 512], F32, tag="y_ps")
                for sc in range(NSC):
                    nc.tensor.matmul(y_ps,
                                     ctx[:, 2 * sc:2 * sc + 2, qs],
                                     wo_sb[sc][:, :, nsl], perf_mode=DR,
                                     start=(sc == 0), stop=(sc == NSC - 1))
                nc.vector.tensor_add(out=y1[:, nsl], in0=y_ps,
                                     in1=xr[:, nsl])  # resid (+bo)
                nc.vector.bn_stats(out=stats[:, no, :], in_=y1g[:, no, :])
            nc.vector.bn_aggr(out=mv_st[:, qc, :], in_=stats)

        def ln_finish(qc, y1):
            qs = slice(qc * 128, (qc + 1) * 128)
            std = ln_pool.tile([128, 1], F32, tag="std")
            nc.scalar.activation(out=std, in_=mv_st[:, qc, 1:2],
                                 func=AF.Sqrt, bias=eps_t)
            rstd = ln_pool.tile([128, 1], F32, tag="rstd")
            nc.vector.reciprocal(out=rstd, in_=std)
            # y2 = (y1 - mu)*rstd split across DVE and ScalarE
            # (scalar form: y1*rstd + (-mu*rstd))
            nmr = ln_pool.tile([128, 1], F32, tag="nmr")
            nc.vector.tensor_scalar(out=nmr, in0=rstd,
                                    scalar1=mv_st[:, qc, 0:1],
                                    scalar2=-1.0,
                                    op0=ALU.mult, op1=ALU.mult)
            y2 = y3_pool.tile([128, D], F32, tag="y2")
            nc.vector.tensor_scalar(out=y2[:, 0:512], in0=y1[:, 0:512],
                                    scalar1=mv_st[:, qc, 0:1],
                                    scalar2=rstd, op0=ALU.subtract,
                                    op1=ALU.mult)
            nc.scalar.activation(out=y2[:, 512:D], in_=y1[:, 512:D],
                                 func=AF.Identity, scale=rstd, bias=nmr)
            if apply_gb:
                nc.vector.tensor_mul(out=y2, in0=y2, in1=gb[0])
                nc.vector.tensor_add(out=y2, in0=y2, in1=gb[1])
            nc.sync.dma_start(out=y[qs, :], in_=y2)

        # ---- background task schedule (drip-fed into the attention loop).
        # ISSUE-ORDER deadlines (Tile derives deps from program order, so a
        # consumer must be issued after its producer): pass qp / head h
        # scores use K[h] (nt cols) and Q[h][:, qp*512:]; pv(s) (issued in
        # super s) reads VT[.][:, 2s:2s+2, :]; out_proj(qc<4) runs in pass 1
        # and reads every head's pass-0 ctx.
        K = lambda h, nt: (lambda: k_task(h, nt))
        Q = lambda h, nt: (lambda: q_task(h, nt))
        V = lambda b, fh: (lambda: v_task(b, fh))
        OP = lambda qc: (lambda: op_mm(qc, y1_st[qc]))
        bg_step = {
            # pass 0 head 0 pops 3/super; VA pair (2s+2, 2s+3) lands one
            # super ahead of pv(s+1)'s issue; K(0,nt) before super 2nt-1.
            (0, 0): [V(2, 0), V(3, 0), K(0, 1),
                     V(4, 0), V(5, 0), K(0, 2),
                     V(6, 0), V(7, 0), K(0, 3),
                     V(8, 0), V(9, 0), K(1, 0),
                     V(10, 0), V(11, 0), K(1, 1),
                     V(12, 0), V(13, 0), K(1, 2),
                     V(14, 0), V(15, 0), K(1, 3),
                     Q(1, 0), K(2, 0), K(2, 1)],
            (0, 1): [K(2, 2), K(2, 3), Q(2, 0)]
                    + [V(b, 1) for b in range(10)]
                    + [K(3, 0), K(3, 1), K(3, 2)],
            (0, 2): [K(3, 3), Q(3, 0)]
                    + [V(b, 1) for b in range(10, 16)]
                    + [K(4, 0), K(4, 1), K(4, 2), K(4, 3), Q(4, 0),
                       K(5, 0), K(5, 1), K(5, 2)],
            (0, 3): [K(5, 3), Q(5, 0),
                     K(6, 0), K(6, 1), K(6, 2), K(6, 3), Q(6, 0),
                     K(7, 0), K(7, 1), K(7, 2), K(7, 3), Q(7, 0),
                     Q(0, 1)],
            # pass 1 is exp-bound with an idle PE: fill it with the
            # deferred second-half Q projections and the out-projections
            # of the already-complete q-half 0
            (1, 0): [Q(1, 1), OP(0)], (1, 1): [Q(2, 1), OP(1)],
            (1, 2): [Q(3, 1), OP(2)], (1, 3): [Q(4, 1), OP(3)],
            (1, 4): [Q(5, 1)], (1, 5): [Q(6, 1)], (1, 6): [Q(7, 1)],
        }
        bg = []
        bg_i = 0

        def pops(n):
            nonlocal bg_i
            for _ in range(n):
                if bg_i < len(bg):
                    bg[bg_i]()
                    bg_i += 1

        # ---- prologue: just enough for pass 0 head 0 to start ------------
        k_task(0, 0)
        q_task(0, 0)
        v_task(0, 0)
        v_task(1, 0)

        # ---- attention: two query passes of 512 --------------------------
        for qp in range(2):
            qsl = slice(qp * 512, (qp + 1) * 512)
            for h in range(H):
                bg += bg_step.get((qp, h), [])
                pop_rate = 3 if (qp, h) == (0, 0) else (2 if qp == 0 else 1)
                if (qp, h) == (0, 2):
                    for sc in range(NSC):
                        nc.sync.dma_start(out=wo_sb[sc], in_=Wo[sc])

                ctx_ps = ctx_psum.tile([128, 512], F32, tag="ctx_ps")
                sum_ps = sum_psum.tile([1, 512], F32, tag="sum_ps")

                def scores_exp(s):
                    pt = pt_pool.tile([128, 2, 512], FP8, tag="pt", name="pt")
                    for i in range(2):
                        kc = 2 * s + i
                        ks = slice(kc * 128, (kc + 1) * 128)
                        sp = scores_ps.tile([128, 512], F32, tag="sp",
                                            name="sp")
                        nc.tensor.matmul(sp, KT[h][:, ks], QT[h][:, qsl],
                                         start=True, stop=True)
                        if kc % 8 in (1, 3, 5):
                            # DVE "Schraudolph" exp: e4m3 bits of exp(y)
                            # ~= uint8(y*8*log2e + 56.5) (PWL between
                            # powers of 2, error ~ e4m3 quantization)
                            nc.vector.tensor_scalar(
                                out=pt[:, i, :].bitcast(mybir.dt.uint8),
                                in0=sp, scalar1=SCH_A, scalar2=SCH_B,
                                op0=ALU.mult, op1=ALU.add)
                        else:
                            nc.scalar.activation(out=pt[:, i, :], in_=sp,
                                                 func=AF.Exp, scale=ISCALE)
                    return pt

                def pv(s, pt):
                    nc.tensor.matmul(ctx_ps, VT[:, h, 2 * s:2 * s + 2, :],
                                     pt, perf_mode=DR,
                                     start=(s == 0), stop=(s == SC - 1))
                    nc.tensor.matmul(sum_ps, ones2, pt, perf_mode=DR,
                                     start=(s == 0), stop=(s == SC - 1))

                # software pipeline: PV lags scores/exp by one super-chunk
                pt_cur = scores_exp(0)
                for s in range(SC):
                    pt_next = scores_exp(s + 1) if s + 1 < SC else None
                    pv(s, pt_cur)
                    pops(pop_rate)
                    pt_cur = pt_next

                # normalize: ctx[h, qsl] = ctx_ps / sum + bv. The psum
                # tiles are freed by quick copies so the next head's PV /
                # ones chains (same single-buffered banks) don't wait on
                # the full reciprocal+broadcast chain.
                # (bv is folded into xq on the host via bv @ Wo)
                ctxs = sums_pool.tile([128, 512], F32, tag="ctxs")
                nc.vector.tensor_copy(out=ctxs, in_=ctx_ps)
                rsum = sums_pool.tile([1, 512], F32, tag="rsum")
                nc.vector.reciprocal_approx_fast(out=rsum, in_=sum_ps)
                rsum_b = sums_pool.tile([128, 512], F32, tag="rsum_b")
                nc.gpsimd.partition_broadcast(rsum_b, rsum, channels=128)
                nc.vector.tensor_mul(out=ctx[:, h, qsl], in0=ctxs,
                                     in1=rsum_b)

        # ---- tail: remaining out-projection blocks with LN finishes
        # interleaved (all exps are done: one Exp->Sqrt table swap); the
        # deferred finishes of the background blocks fill the gaps
        y1_t = [y3_pool.tile([128, D], F32, tag=f"y1{qc % 2}", name="y1")
                for qc in range(4, QC)]
        for qc in range(4, QC):
            op_mm(qc, y1_t[qc - 4])
            if qc >= 5:
                ln_finish(qc - 5, y1_st[qc - 5])
        ln_finish(3, y1_st[3])
        for qc in range(4, QC):
            ln_finish(qc, y1_t[qc - 4])


def build(apply_gb=True):
    nc = bacc.Bacc("TRN2", target_bir_lowering=False, debug=False,
                   enable_asserts=False, num_devices=N_CORES)
    ap = {}
    ap['xq'] = nc.dram_tensor("xq", [TQ, D], F32, kind="ExternalInput").ap()
    ap['xT'] = nc.dram_tensor("xT", [NSC, 128, 2, T], FP8,
                              kind="ExternalInput").ap()
    ap['bq'] = nc.dram_tensor("bq", [D], F32, kind="ExternalInput").ap()
    for w in ('Wq', 'Wk', 'Wv', 'Wo'):
        ap[w] = nc.dram_tensor(w, [NSC, 128, 2, D], FP8,
                               kind="ExternalInput").ap()
    ap['gamma'] = nc.dram_tensor("gamma", [D], F32, kind="ExternalInput").ap()
    ap['beta'] = nc.dram_tensor("beta", [D], F32, kind="ExternalInput").ap()
    ap['y'] = nc.dram_tensor("y", [TQ, D], F32, kind="ExternalOutput").ap()

    with tile.TileContext(nc) as tc, contextlib.ExitStack() as es:
        _body(nc, tc, ap, es, apply_gb)
    nc.compile()
    return nc


def make_in_maps(inputs):
    """Per-core input maps; x token-rotated so q tokens come first."""
    f32 = {k: np.ascontiguousarray(np.asarray(v, dtype=np.float32))
           for k, v in inputs.items()}
    shared = {k: f32[k] for k in ('bq', 'gamma', 'beta')}

    def pair4(a):  # [D, N] -> [NSC, 128, 2, N] DoubleRow pair layout
        return np.ascontiguousarray(
            a.astype(F8).reshape(NSC, 2, 128, a.shape[1]).transpose(
                0, 2, 1, 3))

    for w in ('Wq', 'Wk', 'Wv', 'Wo'):
        shared[w] = pair4(f32[w])
    x = f32['x']
    # fold both the out-proj bias and the V bias into the residual:
    # (ctx0 + bv) @ Wo + bo + x = ctx0 @ Wo + (x + bo + bv@Wo)
    resid = f32['bo'] + f32['bv'] @ f32['Wo']
    in_maps = []
    for core in range(N_CORES):
        b, g = divmod(core, 2)
        xr = np.roll(x[b], -TQ * g, axis=0)
        in_maps.append({'xq': np.ascontiguousarray(xr[:TQ] + resid),
                        'xT': pair4(xr.T),
                        **shared})
    return in_maps


_NC = None
_NC_GB = None


def kernel(**inputs):
    global _NC, _NC_GB
    apply_gb = not (np.all(np.asarray(inputs['gamma']) == 1.0)
                    and np.all(np.asarray(inputs['beta']) == 0.0))
    if _NC is None or _NC_GB != apply_gb:
        _NC = build(apply_gb)
        _NC_GB = apply_gb
    in_maps = make_in_maps(inputs)
    res = bass_utils.run_bass_kernel_spmd(_NC, in_maps,
                                          core_ids=list(range(N_CORES)))
    out = np.empty((B, T, D), dtype=np.float32)
    for core in range(N_CORES):
        b, g = divmod(core, 2)
        out[b, TQ * g:TQ * (g + 1)] = res.results[core]['y']
    return out
